# revision 22
# baseline (speedup 1.0000x reference)
"""GAT 3-layer multi-head GNN (residual+BN) on 8 TRN2 NeuronCores via Bass.

Sharding: nodes + incoming edges partitioned by dst across 8 cores; weights
replicated; per-layer node-feature tables in DRAM gathered per edge with
dma_gather (int16 idx, 4 src-chunks of 25088 rows); segment softmax and
aggregation via DVE iota-compare masks + TensorE segment matmuls into PSUM
(no scatter); AllGather of node features between layers; AllReduce pooling.

Edge order per core: (src_chunk, dst_window, dst). Per-(chunk,window) group
sizes are padded to the max across cores so one static program serves all 8
cores (SPMD). All index prep is host-side integer work; float math on device.
"""
import sys
import numpy as np

sys.path.insert(0, "/opt/trn_rl_repo")

N, E, G, H = 100000, 1600000, 256, 2
NC = 8
NPC = N // NC                 # 12500
NWIN = 98
NPCP = NWIN * 128             # 12544
NTAB = NC * NPCP              # 100352
NCHUNK = 4
CHROWS = NTAB // NCHUNK       # 25088
GCALL = 1024                  # idx per dma_gather call
STE = 6 * GCALL               # supertile edges (48 blocks)
EPS = 1e-5

LAYERS = [  # (F_in, C, table_cols)
    (5, 32, 128),
    (64, 64, 256),
    (128, 64, 256),
]

_STATE = {}


def _wrap16(idx2d):
    nc_, n = idx2d.shape
    w = idx2d.reshape(nc_, n // 16, 16).transpose(0, 2, 1)
    return np.ascontiguousarray(np.tile(w, (1, 8, 1)).astype(np.int16))


def _host_prep(edge_index, batch):
    src = np.concatenate([edge_index[0].astype(np.int64), np.arange(N, dtype=np.int64)])
    dst = np.concatenate([edge_index[1].astype(np.int64), np.arange(N, dtype=np.int64)])
    core = dst // NPC
    dst_local = dst - core * NPC
    src_perm = src + 44 * (src // NPC)

    per_core = []
    cnt = np.zeros((NC, NCHUNK, NWIN), np.int64)
    for k in range(NC):
        sel = core == k
        s, dl = src_perm[sel], dst_local[sel]
        ch = s // CHROWS
        w = dl // 128
        order = np.lexsort((dl, w, ch))
        s, dl, ch, w = s[order], dl[order], ch[order], w[order]
        per_core.append((s, dl, ch, w))
        np.add.at(cnt[k], (ch, w), 1)

    gsz = ((cnt.max(axis=0) + 127) // 128) * 128        # [NCHUNK, NWIN]
    chunk_tot = gsz.sum(axis=1)
    chunk_pad = ((chunk_tot + STE - 1) // STE) * STE
    tail = chunk_pad - chunk_tot
    epad = int(chunk_pad.sum())

    idx16 = np.zeros((NC, epad), np.int16)
    dstrel = np.full((NC, epad), -1.0, np.float32)
    for k in range(NC):
        s, dl, ch, w = per_core[k]
        pos = ptr = 0
        for c in range(NCHUNK):
            for wi in range(NWIN):
                g, n = int(gsz[c, wi]), int(cnt[k, c, wi])
                idx16[k, pos:pos + n] = (s[ptr:ptr + n] - c * CHROWS).astype(np.int16)
                dstrel[k, pos:pos + n] = (dl[ptr:ptr + n] - 128 * wi).astype(np.float32)
                ptr += n
                pos += g
            pos += int(tail[c])
        assert ptr == len(s) and pos == epad

    # group schedule: (window, n_blocks) in edge order; chunk tails are inert
    # groups on the last window (masks all-zero there).
    groups = []
    st_chunk = []                      # chunk id per supertile
    for c in range(NCHUNK):
        for wi in range(NWIN):
            if gsz[c, wi]:
                groups.append((wi, int(gsz[c, wi]) // 128))
        if tail[c]:
            groups.append((NWIN - 1, int(tail[c]) // 128))
        st_chunk += [c] * (int(chunk_pad[c]) // STE)

    # per-block: (window, is_first_of_group, is_last_of_group)
    blocksched = []
    for wi, nb in groups:
        for j in range(nb):
            blocksched.append((wi, j == 0, j == nb - 1))
    assert len(blocksched) == epad // 128

    batchv = np.full((NC, NPCP), -1.0, np.float32)
    b = np.asarray(batch, np.int64)
    for k in range(NC):
        batchv[k, :NPC] = b[k * NPC:(k + 1) * NPC].astype(np.float32)

    import ml_dtypes
    meta_drrep = np.repeat(dstrel.astype(ml_dtypes.bfloat16)[:, None, :], 128, axis=1)
    return {
        "drrep": meta_drrep,
        "epad": epad,
        "st_chunk": st_chunk,
        "blocksched": blocksched,
        "idx16w": _wrap16(idx16),
        "dstrel_col": np.ascontiguousarray(
            dstrel.reshape(NC, epad // 128, 128).transpose(0, 2, 1)),   # [NC,128,nblk]
        "batchv": np.ascontiguousarray(
            batchv.reshape(NC, NWIN, 128).transpose(0, 2, 1)),           # [NC,128,NWIN]
    }


def _build_nc(meta):
    from concourse import bass, bacc, mybir
    import concourse.tile as tile
    from concourse.masks import make_identity

    F32, BF16, I16 = mybir.dt.float32, mybir.dt.bfloat16, mybir.dt.int16
    AOP = mybir.AluOpType
    AF = mybir.ActivationFunctionType

    epad = meta["epad"]
    nblk_tot = epad // 128
    nst = epad // STE
    st_chunk = meta["st_chunk"]
    blocksched = meta["blocksched"]

    nc = bacc.Bacc("TRN2", target_bir_lowering=False, debug=False, num_devices=NC)

    def inp(name, shape, dt=F32):
        return nc.dram_tensor(name, shape, dt, kind="ExternalInput")

    xT = inp("xT", [5, NTAB])
    xTown = inp("xTown", [5, NPCP])
    idx16_d = inp("idx16", [128, epad // 16], I16)
    drcol_d = inp("drcol", [128, nblk_tot])
    drrep_d = inp("drrep", [128, epad], BF16)
    batchv_d = inp("batchv", [128, NWIN])
    iota_col_d = inp("iota_col", [128, 1])
    iota_mat_d = inp("iota_mat", [128, GCALL], BF16)

    ws = {}
    for l in (1, 2, 3):
        Fi, C, TC = LAYERS[l - 1]
        ws[f"w{l}"] = inp(f"w{l}", [Fi, 2 * C])
        ws[f"w{l}T"] = inp(f"w{l}T", [2 * C, Fi])
        ws[f"ast{l}"] = inp(f"ast{l}", [2 * C, 2])   # [:,0]=a_src heads stacked, [:,1]=a_dst
        for nm in ("b", "g", "be", "m", "v", "pb"):
            ws[f"{nm}{l}"] = inp(f"{nm}{l}", [128, 2 * C])
        ws[f"pw{l}"] = inp(f"pw{l}", [Fi, 2 * C])
    ws["fw"] = inp("fw", [128, 32])
    for nm, d in (("fb", 32), ("g4", 32), ("be4", 32), ("m4", 32), ("v4", 32),
                  ("l1b", 32), ("l2b", 10)):
        ws[nm] = inp(nm, [128, d])
    ws["l1w"] = inp("l1w", [32, 32])
    ws["l2w"] = inp("l2w", [32, 10])

    out_d = nc.dram_tensor("out", [256, 10], F32, kind="ExternalOutput")

    xTbf_d = nc.dram_tensor("xTbf", [5, NTAB], BF16)
    xTownbf_d = nc.dram_tensor("xTownbf", [5, NPCP], BF16)
    table_d = nc.dram_tensor("table", [NTAB, 256], BF16)
    shard_d = nc.dram_tensor("shard", [128, NPCP], BF16)
    gath_d = nc.dram_tensor("gath", [128 * NC, NPCP], BF16, addr_space="Shared")
    poolin_d = nc.dram_tensor("poolin", [128, 2, 132], F32)
    poolout_d = nc.dram_tensor("poolout", [128, 2, 132], F32, addr_space="Shared")
    proj_d = nc.dram_tensor("proj", [128, NWIN * 128], BF16)

    def bc(ap, shape):
        if ap.shape[0] == 1 and shape[0] != 1:
            ap = ap.partition_broadcast(shape[0]).squeeze()
        while ap.ndim < len(shape):
            ap = ap.unsqueeze(1)
        return ap.to_broadcast(shape) if list(ap.shape) != list(shape) else ap

    with tile.TileContext(nc) as tc:
        with tc.tile_pool(name="const", bufs=1) as cpool, \
             tc.tile_pool(name="accp", bufs=1) as apool, \
             tc.tile_pool(name="gat", bufs=2) as gpool, \
             tc.tile_pool(name="msk", bufs=2) as mpool, \
             tc.tile_pool(name="sml", bufs=3) as spool, \
             tc.tile_pool(name="wts", bufs=1) as wpool, \
             tc.tile_pool(name="tbl", bufs=2) as tpool, \
             tc.tile_pool(name="psA", bufs=3, space="PSUM") as psA, \
             tc.tile_pool(name="psB", bufs=3, space="PSUM") as psB, \
             tc.tile_pool(name="psC", bufs=2, space="PSUM") as psC:

            ident = cpool.tile([128, 128], F32, tag="ident")
            make_identity(nc, ident[:])
            identb = cpool.tile([128, 128], BF16, tag="identb")
            nc.vector.tensor_copy(out=identb[:], in_=ident[:])
            iota_col = cpool.tile([128, 1], F32, tag="iotac")
            nc.sync.dma_start(out=iota_col[:], in_=iota_col_d.ap())
            iota_mat = cpool.tile([128, GCALL], BF16, tag="iotam")
            nc.sync.dma_start(out=iota_mat[:], in_=iota_mat_d.ap())

            drcol = cpool.tile([128, nblk_tot], F32, tag="drcol")
            nc.sync.dma_start(out=drcol[:], in_=drcol_d.ap())
            batchv = cpool.tile([128, NWIN], F32, tag="batchv")
            nc.sync.dma_start(out=batchv[:], in_=batchv_d.ap())

            acc = apool.tile([128, NWIN, 132], F32, tag="acc")
            xsh = apool.tile([128, NWIN, 132], BF16, tag="xsh")
            c_sb = apool.tile([128, NWIN, 2], BF16, tag="c_sb")

            # f32->bf16 casts of x inputs (SWDGE cast-DMA)
            nc.gpsimd.dma_start(out=xTbf_d.ap(), in_=xT.ap())
            nc.gpsimd.dma_start(out=xTownbf_d.ap(), in_=xTown.ap())

            for l in (1, 2, 3):
                Fi, C, TC = LAYERS[l - 1]
                Fo = 2 * C

                # ---- weights: W_aug bf16 [Fi, Fo+4], proj bf16, bn vectors ----
                wfull = wpool.tile([128, 260], F32, tag="waug")
                nc.sync.dma_start(out=wfull[:Fi, 0:Fo], in_=ws[f"w{l}"].ap())
                wT = wpool.tile([128, 128], F32, tag="wT")
                nc.sync.dma_start(out=wT[:Fo, 0:Fi], in_=ws[f"w{l}T"].ap())
                aT = wpool.tile([128, 2], F32, tag="aT")
                nc.sync.dma_start(out=aT[:Fo, 0:2], in_=ws[f"ast{l}"].ap())
                ps_v = psC.tile([128, 4], F32, tag="ps_small")
                for j in range(2):          # 0: a_src, 1: a_dst
                    for h in range(H):
                        nc.tensor.matmul(
                            out=ps_v[:Fi, 2 * j + h:2 * j + h + 1],
                            lhsT=wT[h * C:(h + 1) * C, 0:Fi],
                            rhs=aT[h * C:(h + 1) * C, j:j + 1],
                            start=True, stop=True)
                nc.vector.tensor_copy(out=wfull[:Fi, Fo:Fo + 4], in_=ps_v[:Fi, :])
                wbf = wpool.tile([128, 260], BF16, tag="wbf")
                nc.vector.tensor_copy(out=wbf[:Fi, 0:Fo + 4], in_=wfull[:Fi, 0:Fo + 4])
                pwbf = wpool.tile([128, 256], BF16, tag="pwbf")
                pwf = wpool.tile([128, 256], F32, tag="pwf")
                nc.sync.dma_start(out=pwf[:Fi, 0:Fo], in_=ws[f"pw{l}"].ap())
                nc.vector.tensor_copy(out=pwbf[:Fi, 0:Fo], in_=pwf[:Fi, 0:Fo])

                # bn affine: s = g / sqrt(v+eps); b2 = be - s*m  (+ gat bias b)
                bnrow = wpool.tile([128, 256], F32, tag="bnrow")     # [s | b2]
                vrow = spool.tile([128, 128], F32, tag="vrow")
                nc.sync.dma_start(out=vrow[:, 0:Fo], in_=ws[f"v{l}"].ap())
                nc.vector.tensor_scalar(vrow[:, 0:Fo], vrow[:, 0:Fo], EPS, None, AOP.add)
                nc.scalar.activation(out=vrow[:, 0:Fo], in_=vrow[:, 0:Fo], func=AF.Sqrt)
                nc.vector.reciprocal(out=bnrow[:, 0:Fo], in_=vrow[:, 0:Fo])
                grow = spool.tile([128, 128], F32, tag="grow")
                nc.sync.dma_start(out=grow[:, 0:Fo], in_=ws[f"g{l}"].ap())
                nc.vector.tensor_tensor(out=bnrow[:, 0:Fo], in0=bnrow[:, 0:Fo],
                                        in1=grow[:, 0:Fo], op=AOP.mult)
                mrow = spool.tile([128, 128], F32, tag="mrow")
                nc.sync.dma_start(out=mrow[:, 0:Fo], in_=ws[f"m{l}"].ap())
                # m' = m - b  (gat bias folds into bn input: bn(x+b) = s*x + (be - s*(m-b)))
                brow = spool.tile([128, 128], F32, tag="brow")
                nc.sync.dma_start(out=brow[:, 0:Fo], in_=ws[f"b{l}"].ap())
                nc.vector.tensor_tensor(out=mrow[:, 0:Fo], in0=mrow[:, 0:Fo],
                                        in1=brow[:, 0:Fo], op=AOP.subtract)
                nc.vector.tensor_tensor(out=mrow[:, 0:Fo], in0=mrow[:, 0:Fo],
                                        in1=bnrow[:, 0:Fo], op=AOP.mult)
                berow = spool.tile([128, 128], F32, tag="berow")
                nc.sync.dma_start(out=berow[:, 0:Fo], in_=ws[f"be{l}"].ap())
                nc.vector.tensor_tensor(out=bnrow[:, 128:128 + Fo], in0=berow[:, 0:Fo],
                                        in1=mrow[:, 0:Fo], op=AOP.subtract)
                pbrow = wpool.tile([128, 256], F32, tag="pbrow")
                nc.sync.dma_start(out=pbrow[:, 0:Fo], in_=ws[f"pb{l}"].ap())

                # ---- node table build ----
                for cs in range(NC):
                    for tb in range(7):       # 7 * 14 = 98 windows
                        xsl = tpool.tile([128, 14 * 128], BF16, tag="xsl")
                        c0 = tb * 14 * 128
                        if l == 1:
                            nc.sync.dma_start(
                                out=xsl[:5, :],
                                in_=xTbf_d.ap()[:, cs * NPCP + c0: cs * NPCP + c0 + 14 * 128])
                        else:
                            nc.sync.dma_start(
                                out=xsl[:Fi, :],
                                in_=gath_d.ap()[cs * 128:cs * 128 + Fi,
                                                c0:c0 + 14 * 128])
                        for nb in range(14):
                            ps_h = psA.tile([128, 260], F32, tag="ps_h")
                            nc.tensor.matmul(
                                out=ps_h[:, 0:Fo + 4],
                                lhsT=xsl[:Fi, nb * 128:(nb + 1) * 128],
                                rhs=wbf[:Fi, 0:Fo + 4], start=True, stop=True)
                            hrow = tpool.tile([128, 256], BF16, tag="hrow")
                            nc.scalar.copy(out=hrow[:, 0:Fo + 2], in_=ps_h[:, 0:Fo + 2])
                            r0 = cs * NPCP + c0 + nb * 128
                            nc.sync.dma_start(
                                out=table_d.ap()[r0:r0 + 128, 0:Fo + 2],
                                in_=hrow[:, 0:Fo + 2])

                # ---- own shard: e_dst cols + residual projection ----
                for tb in range(7):
                    xsl = tpool.tile([128, 14 * 128], BF16, tag="xslo")
                    c0 = tb * 14 * 128
                    src_ap = (xTownbf_d if l == 1 else shard_d).ap()
                    nc.sync.dma_start(out=xsl[:Fi, :], in_=src_ap[0:Fi, c0:c0 + 14 * 128])
                    for nb in range(14):
                        nwb = tb * 14 + nb
                        ps_h = psA.tile([128, 260], F32, tag="ps_h")
                        nc.tensor.matmul(out=ps_h[:, 0:Fo + 4],
                                         lhsT=xsl[:Fi, nb * 128:(nb + 1) * 128],
                                         rhs=wbf[:Fi, 0:Fo + 4], start=True, stop=True)
                        nc.vector.tensor_copy(out=c_sb[:, nwb, :], in_=ps_h[:, Fo + 2:Fo + 4])
                        ps_p = psA.tile([128, 260], F32, tag="ps_h", name="ps_p")
                        nc.tensor.matmul(out=ps_p[:, 0:Fo],
                                         lhsT=xsl[:Fi, nb * 128:(nb + 1) * 128],
                                         rhs=pwbf[:Fi, 0:Fo], start=True, stop=True)
                        prt = tpool.tile([128, 128], BF16, tag="prt")
                        nc.scalar.copy(out=prt[:, 0:Fo], in_=ps_p[:, 0:Fo])
                        nc.sync.dma_start(out=proj_d.ap()[:, nwb * 128:nwb * 128 + Fo],
                                          in_=prt[:, 0:Fo])

                nc.vector.memset(acc[:, :, 0:132], 0.0)

                # ---- edge pipeline ----
                blk = 0
                ps_seg = None
                for st in range(nst):
                    ch = st_chunk[st]
                    hg = gpool.tile([128, 48, TC], BF16, tag="hg")
                    idxt = spool.tile([128, STE // 16], I16, tag="idxt")
                    nc.sync.dma_start(
                        out=idxt[:],
                        in_=idx16_d.ap()[:, st * (STE // 16):(st + 1) * (STE // 16)])
                    ps_c = psC.tile([128, 48, 2], F32, tag="ps_small")
                    for t in range(6):
                        eb = st * STE + t * GCALL
                        nc.gpsimd.dma_gather(
                            out_ap=hg[:, 8 * t:8 * t + 8, :],
                            in_ap=table_d.ap()[ch * CHROWS:(ch + 1) * CHROWS, 0:TC],
                            idxs_ap=idxt[:, t * 64:(t + 1) * 64],
                            num_idxs=GCALL, num_idxs_reg=GCALL, elem_size=TC, elem_step=256)
                        drrep = mpool.tile([128, GCALL], BF16, tag="drrep")
                        nc.sync.dma_start(out=drrep[:], in_=drrep_d.ap()[:, eb:eb + GCALL])
                        m_dT = mpool.tile([128, 8, 128], BF16, tag="mdT")
                        nc.vector.tensor_scalar(
                            m_dT[:].rearrange("p a b -> p (a b)"),
                            drrep[:], iota_col[:], None, AOP.is_equal)
                        for jj in range(8):
                            j = 8 * t + jj
                            nc.tensor.matmul(out=ps_c[:, j, :], lhsT=m_dT[:, jj, :],
                                             rhs=c_sb[:, blocksched[blk + j][0], :],
                                             start=True, stop=True)
                    alf = spool.tile([128, 48, 2], F32, tag="alf")
                    nc.vector.tensor_copy(out=alf[:], in_=hg[:, :, Fo:Fo + 2])
                    nc.vector.tensor_tensor(out=alf[:], in0=alf[:], in1=ps_c[:], op=AOP.add)
                    nc.scalar.activation(out=alf[:], in_=alf[:], func=AF.Lrelu, alpha=0.2)
                    nc.scalar.activation(out=alf[:], in_=alf[:], func=AF.Exp)
                    nc.vector.tensor_copy(out=hg[:, :, Fo:Fo + 2], in_=alf[:])
                    for h in range(H):
                        nc.vector.tensor_tensor(
                            out=hg[:, :, h * C:(h + 1) * C],
                            in0=hg[:, :, h * C:(h + 1) * C],
                            in1=hg[:, :, Fo + h:Fo + h + 1].to_broadcast([128, 48, C]),
                            op=AOP.mult)
                    for t in range(6):
                        m_ed = mpool.tile([128, 8, 128], BF16, tag="med")
                        nc.vector.tensor_tensor(
                            out=m_ed[:],
                            in0=iota_mat[:].rearrange("p (a b) -> p a b", b=128),
                            in1=drcol[:, st * 48 + 8 * t:st * 48 + 8 * t + 8]
                                .unsqueeze(-1).to_broadcast([128, 8, 128]),
                            op=AOP.is_equal)
                        for jj in range(8):
                            j = 8 * t + jj
                            wi, first, last = blocksched[blk + j]
                            if first:
                                ps_seg = psB.tile([128, 132], F32, tag="ps_seg")
                            nc.tensor.matmul(out=ps_seg[:, 0:Fo + 2], lhsT=m_ed[:, jj, :],
                                             rhs=hg[:, j, 0:Fo + 2], start=first, stop=last)
                            if last:
                                nc.vector.tensor_tensor(
                                    out=acc[:, wi, 0:Fo + 2], in0=acc[:, wi, 0:Fo + 2],
                                    in1=ps_seg[:, 0:Fo + 2], op=AOP.add)
                    blk += 48

                # ---- finalize: normalize, bn, relu, +proj ----
                nc.vector.tensor_scalar(acc[:, :, Fo:Fo + 2], acc[:, :, Fo:Fo + 2],
                                        1e-16, None, AOP.add)
                rc = spool.tile([128, NWIN, 2], F32, tag="rc")
                nc.vector.reciprocal(out=rc[:], in_=acc[:, :, Fo:Fo + 2])
                for h in range(H):
                    nc.vector.tensor_tensor(
                        out=acc[:, :, h * C:(h + 1) * C],
                        in0=acc[:, :, h * C:(h + 1) * C],
                        in1=rc[:, :, h:h + 1].to_broadcast([128, NWIN, C]), op=AOP.mult)
                nc.vector.tensor_tensor(
                    out=acc[:, :, 0:Fo], in0=acc[:, :, 0:Fo],
                    in1=bnrow[:, 0:Fo].unsqueeze(1).to_broadcast([128, NWIN, Fo]), op=AOP.mult)
                nc.vector.tensor_tensor(
                    out=acc[:, :, 0:Fo], in0=acc[:, :, 0:Fo],
                    in1=bnrow[:, 128:128 + Fo].unsqueeze(1).to_broadcast([128, NWIN, Fo]), op=AOP.add)
                nc.scalar.activation(out=xsh[:, :, 0:Fo], in_=acc[:, :, 0:Fo], func=AF.Relu)
                for hf in range(2):
                    nw2 = NWIN // 2
                    projt = gpool.tile([128, nw2, 128], BF16, tag="hg", name=f"projt{l}{hf}")
                    nc.sync.dma_start(
                        out=projt[:].rearrange("p a b -> p (a b)"),
                        in_=proj_d.ap()[:, hf * nw2 * 128:(hf + 1) * nw2 * 128])
                    nc.vector.tensor_tensor(
                        out=xsh[:, hf * nw2:(hf + 1) * nw2, 0:Fo],
                        in0=xsh[:, hf * nw2:(hf + 1) * nw2, 0:Fo],
                        in1=projt[:, :, 0:Fo], op=AOP.add)
                nc.vector.tensor_tensor(
                    out=xsh[:, :, 0:Fo], in0=xsh[:, :, 0:Fo],
                    in1=pbrow[:, 0:Fo].unsqueeze(1).to_broadcast([128, NWIN, Fo]), op=AOP.add)

                if l < 3:
                    # transpose shard to feat-major, ship, all-gather
                    for nb in range(NWIN):
                        ps_t = psA.tile([128, 260], BF16, tag="ps_h", name="ps_tb")
                        nc.tensor.matmul(out=ps_t[:, 0:128], lhsT=xsh[:, nb, 0:128],
                                         rhs=identb[:], is_transpose=True, start=True, stop=True)
                        tt = tpool.tile([128, 128], BF16, tag="tt")
                        nc.scalar.copy(out=tt[:Fo, :], in_=ps_t[0:Fo, 0:128])
                        nc.sync.dma_start(out=shard_d.ap()[0:Fo, nb * 128:(nb + 1) * 128],
                                          in_=tt[:Fo, :])
                    nc.gpsimd.collective_compute(
                        "AllGather", AOP.bypass, replica_groups=[list(range(NC))],
                        ins=[shard_d.ap()], outs=[gath_d.ap()])

            # ---------- pooling + MLP head ----------
            Fo = 256 if False else 128
            nc.vector.memset(xsh[:, :, 128:129], 1.0)
            bm1 = spool.tile([128, NWIN], F32, tag="bm1")
            nc.vector.tensor_scalar(bm1[:], batchv[:], -128.0, None, AOP.add)
            ps_pool = [psB.tile([128, 132], F32, name=f"pspool{_g}", tag="ps_seg") for _g in range(2)]
            for nb in range(NWIN):
                for gh in range(2):
                    gm = spool.tile([128, 128], BF16, tag="gm")
                    nc.vector.tensor_scalar(
                        gm[:], iota_mat[:, 0:128],
                        (batchv if gh == 0 else bm1)[:, nb:nb + 1], None, AOP.is_equal)
                    nc.tensor.matmul(out=ps_pool[gh][:, 0:129], lhsT=gm[:],
                                     rhs=xsh[:, nb, 0:129],
                                     start=(nb == 0), stop=(nb == NWIN - 1))
            plin = spool.tile([128, 2, 132], F32, tag="plin")
            for gh in range(2):
                nc.vector.tensor_copy(out=plin[:, gh, 0:129], in_=ps_pool[gh][:, 0:129])
            nc.sync.dma_start(out=poolin_d.ap().rearrange("p a f -> p (a f)"),
                              in_=plin[:].rearrange("p a f -> p (a f)"))
            nc.gpsimd.collective_compute(
                "AllReduce", AOP.add, replica_groups=[list(range(NC))],
                ins=[poolin_d.ap()], outs=[poolout_d.ap()])
            pl = spool.tile([128, 2, 132], F32, tag="pl")
            nc.sync.dma_start(out=pl[:].rearrange("p a f -> p (a f)"),
                              in_=poolout_d.ap().rearrange("p a f -> p (a f)"))
            # mean
            cnt_t = spool.tile([128, 2, 1], F32, tag="cnt")
            nc.vector.tensor_scalar(cnt_t[:].rearrange("p a b -> p (a b)"),
                                    pl[:, :, 128:129].rearrange("p a b -> p (a b)"),
                                    1.0, None, AOP.max)
            rcc = spool.tile([128, 2, 1], F32, tag="rcc")
            nc.vector.reciprocal(out=rcc[:].rearrange("p a b -> p (a b)"),
                                 in_=cnt_t[:].rearrange("p a b -> p (a b)"))
            nc.vector.tensor_tensor(out=pl[:, :, 0:128], in0=pl[:, :, 0:128],
                                    in1=rcc[:].to_broadcast([128, 2, 128]), op=AOP.mult)

            # z1 = relu(bn4(pooled @ fw + fb))
            fw_t = spool.tile([128, 32], F32, tag="fw")
            nc.sync.dma_start(out=fw_t[:], in_=ws["fw"].ap())
            z1 = spool.tile([128, 2, 32], F32, tag="z1")
            for gh in range(2):
                ps_t = psA.tile([128, 260], F32, tag="ps_h", name="ps_plT")
                nc.tensor.matmul(out=ps_t[:, 0:128], lhsT=pl[:, gh, 0:128], rhs=ident[:],
                                 is_transpose=True, start=True, stop=True)
                plT = spool.tile([128, 128], F32, tag="plT")
                nc.vector.tensor_copy(out=plT[:], in_=ps_t[:, 0:128])
                ps_z = psC.tile([128, 48, 2], F32, tag="ps_small", name="ps_z1")
                nc.tensor.matmul(out=ps_z[:].rearrange("p a b -> p (a b)")[0:128, 0:32],
                                 lhsT=plT[:], rhs=fw_t[:], start=True, stop=True)
                nc.vector.tensor_copy(out=z1[:, gh, :],
                                      in_=ps_z[:].rearrange("p a b -> p (a b)")[:, 0:32])
            # bn4 affine on device
            s4 = spool.tile([128, 32], F32, tag="s4")
            b4 = spool.tile([128, 32], F32, tag="b4")
            v4r = spool.tile([128, 32], F32, tag="v4r")
            nc.sync.dma_start(out=v4r[:], in_=ws["v4"].ap())
            nc.vector.tensor_scalar(v4r[:], v4r[:], EPS, None, AOP.add)
            nc.scalar.activation(out=v4r[:], in_=v4r[:], func=AF.Sqrt)
            nc.vector.reciprocal(out=s4[:], in_=v4r[:])
            g4r = spool.tile([128, 32], F32, tag="g4r")
            nc.sync.dma_start(out=g4r[:], in_=ws["g4"].ap())
            nc.vector.tensor_tensor(out=s4[:], in0=s4[:], in1=g4r[:], op=AOP.mult)
            m4r = spool.tile([128, 32], F32, tag="m4r")
            nc.sync.dma_start(out=m4r[:], in_=ws["m4"].ap())
            fbr = spool.tile([128, 32], F32, tag="fbr")
            nc.sync.dma_start(out=fbr[:], in_=ws["fb"].ap())
            nc.vector.tensor_tensor(out=m4r[:], in0=m4r[:], in1=fbr[:], op=AOP.subtract)
            nc.vector.tensor_tensor(out=m4r[:], in0=m4r[:], in1=s4[:], op=AOP.mult)
            be4r = spool.tile([128, 32], F32, tag="be4r")
            nc.sync.dma_start(out=be4r[:], in_=ws["be4"].ap())
            nc.vector.tensor_tensor(out=b4[:], in0=be4r[:], in1=m4r[:], op=AOP.subtract)
            nc.vector.tensor_tensor(out=z1[:], in0=z1[:],
                                    in1=s4[:].unsqueeze(1).to_broadcast([128, 2, 32]), op=AOP.mult)
            nc.vector.tensor_tensor(out=z1[:], in0=z1[:],
                                    in1=b4[:].unsqueeze(1).to_broadcast([128, 2, 32]), op=AOP.add)
            nc.scalar.activation(out=z1[:], in_=z1[:], func=AF.Relu)

            # z2 = relu(z1 @ l1w + l1b); out = z2 @ l2w + l2b
            def mlp_step(zin, wname, bname, dout, relu):
                wt = spool.tile([32, 32], F32, tag="wmlp")
                nc.sync.dma_start(out=wt[:32, 0:dout], in_=ws[wname].ap())
                br = spool.tile([128, 32], F32, tag="bmlp")
                nc.sync.dma_start(out=br[:, 0:dout], in_=ws[bname].ap())
                zo = spool.tile([128, 2, 32], F32, tag="zo")
                for gh in range(2):
                    ps_t = psA.tile([128, 260], F32, tag="ps_h", name="ps_mlpT")
                    nc.tensor.matmul(out=ps_t[0:32, 0:128], lhsT=zin[:, gh, 0:32],
                                     rhs=ident[:], is_transpose=True, start=True, stop=True)
                    zT = spool.tile([32, 128], F32, tag="zT")
                    nc.vector.tensor_copy(out=zT[:], in_=ps_t[0:32, 0:128])
                    ps_z = psC.tile([128, 48, 2], F32, tag="ps_small", name="ps_z2")
                    nc.tensor.matmul(out=ps_z[:].rearrange("p a b -> p (a b)")[0:128, 0:dout],
                                     lhsT=zT[:], rhs=wt[:32, 0:dout], start=True, stop=True)
                    nc.vector.tensor_copy(out=zo[:, gh, 0:dout],
                                          in_=ps_z[:].rearrange("p a b -> p (a b)")[:, 0:dout])
                nc.vector.tensor_tensor(out=zo[:, :, 0:dout], in0=zo[:, :, 0:dout],
                                        in1=br[:, 0:dout].unsqueeze(1).to_broadcast([128, 2, dout]),
                                        op=AOP.add)
                if relu:
                    nc.scalar.activation(out=zo[:, :, 0:dout], in_=zo[:, :, 0:dout], func=AF.Relu)
                return zo

            z2 = mlp_step(z1, "l1w", "l1b", 32, True)
            z3 = mlp_step(z2, "l2w", "l2b", 10, False)
            for gh in range(2):
                nc.sync.dma_start(out=out_d.ap()[gh * 128:(gh + 1) * 128, :],
                                  in_=z3[:, gh, 0:10])

    return nc


def _make_in_maps(inputs, meta):
    x = np.asarray(inputs["x"], np.float32)
    xTp = np.zeros((5, NTAB), np.float32)
    xp = x.reshape(NC, NPC, 5)
    for k in range(NC):
        xTp[:, k * NPCP:k * NPCP + NPC] = xp[k].T
    iota_col = np.arange(128, dtype=np.float32).reshape(128, 1)
    import ml_dtypes
    iota_mat = np.tile(np.arange(128, dtype=np.float32), GCALL // 128).reshape(1, GCALL)
    iota_mat = np.repeat(iota_mat, 128, axis=0).astype(ml_dtypes.bfloat16)

    common = {"xT": xTp, "iota_col": iota_col, "iota_mat": iota_mat}
    for l in (1, 2, 3):
        w = np.asarray(inputs[f"w{l}"], np.float32)
        common[f"w{l}"] = w
        common[f"w{l}T"] = np.ascontiguousarray(w.T)
        a_s = np.asarray(inputs[f"as{l}"], np.float32).reshape(-1, 1)
        a_d = np.asarray(inputs[f"ad{l}"], np.float32).reshape(-1, 1)
        common[f"ast{l}"] = np.ascontiguousarray(np.concatenate([a_s, a_d], axis=1))
        for nm in ("b", "g", "be", "m", "v"):
            common[f"{nm}{l}"] = np.repeat(
                np.asarray(inputs[f"{nm}{l}"], np.float32).reshape(1, -1), 128, axis=0)
        common[f"pw{l}"] = np.asarray(inputs[f"p{l}w"], np.float32)
        common[f"pb{l}"] = np.repeat(
            np.asarray(inputs[f"p{l}b"], np.float32).reshape(1, -1), 128, axis=0)
    common["fw"] = np.asarray(inputs["fw"], np.float32)
    for nm, src in (("fb", "fb"), ("g4", "g4"), ("be4", "be4"), ("m4", "m4"),
                    ("v4", "v4"), ("l1b", "l1b"), ("l2b", "l2b")):
        common[nm] = np.repeat(np.asarray(inputs[src], np.float32).reshape(1, -1), 128, axis=0)
    common["l1w"] = np.asarray(inputs["l1w"], np.float32)
    common["l2w"] = np.asarray(inputs["l2w"], np.float32)

    in_maps = []
    for k in range(NC):
        m = dict(common)
        m["xTown"] = np.ascontiguousarray(xTp[:, k * NPCP:(k + 1) * NPCP])
        m["idx16"] = meta["idx16w"][k]
        m["drcol"] = meta["dstrel_col"][k]
        m["drrep"] = meta["drrep"][k]
        m["batchv"] = meta["batchv"][k]
        in_maps.append(m)
    return in_maps


class _Runner:
    def __init__(self, nc, n_cores):
        import jax
        from jax.sharding import Mesh, PartitionSpec
        from jax.experimental.shard_map import shard_map
        from concourse import bass2jax, mybir

        bass2jax.install_neuronx_cc_hook()
        nc.finalize()
        self.n_cores = n_cores
        partition_name = nc.partition_id_tensor.name if nc.partition_id_tensor else None
        in_names, out_names, out_avals, zero_outs = [], [], [], []
        for alloc in nc.m.functions[0].allocations:
            if not isinstance(alloc, mybir.MemoryLocationSet):
                continue
            name = alloc.memorylocations[0].name
            if alloc.kind == "ExternalInput":
                if name != partition_name:
                    in_names.append(name)
            elif alloc.kind == "ExternalOutput":
                shape = tuple(alloc.tensor_shape)
                dtype = mybir.dt.np(alloc.dtype)
                out_names.append(name)
                out_avals.append(jax.core.ShapedArray(shape, dtype))
                zero_outs.append(np.zeros(shape, dtype))
        self.in_names, self.out_names = in_names, out_names
        self.out_avals = out_avals
        n_params, n_outs = len(in_names), len(out_avals)
        self.n_params = n_params
        all_in = list(in_names) + list(out_names)
        if partition_name is not None:
            all_in.append(partition_name)

        def _body(*args):
            operands = list(args)
            if partition_name is not None:
                operands.append(bass2jax.partition_id_tensor())
            return tuple(bass2jax._bass_exec_p.bind(
                *operands, out_avals=tuple(out_avals), in_names=tuple(all_in),
                out_names=tuple(out_names), lowering_input_output_aliases=(),
                sim_require_finite=False, sim_require_nnan=False, nc=nc))

        devices = jax.devices()[:n_cores]
        mesh = Mesh(np.asarray(devices), ("core",))
        self.fn = jax.jit(
            shard_map(_body, mesh=mesh,
                      in_specs=(PartitionSpec("core"),) * (n_params + n_outs),
                      out_specs=(PartitionSpec("core"),) * n_outs, check_rep=False),
            keep_unused=True)
        self.zero_outs = zero_outs

    def prepare(self, in_maps):
        per_core = [[np.asarray(m[nm]) for nm in self.in_names] for m in in_maps]
        args = [np.concatenate([per_core[c][i] for c in range(self.n_cores)], axis=0)
                for i in range(self.n_params)]
        args += [np.zeros((self.n_cores * z.shape[0], *z.shape[1:]), z.dtype)
                 for z in self.zero_outs]
        return args

    def run(self, args):
        import jax
        outs = self.fn(*args)
        jax.block_until_ready(outs)
        return outs

    def result0(self, outs, name="out"):
        i = self.out_names.index(name)
        a = np.asarray(outs[i])
        per = self.out_avals[i].shape[0]
        return a[:per]


def build_state(**inputs):
    meta = _host_prep(np.asarray(inputs["edge_index"]), np.asarray(inputs["batch"]))
    nc = _build_nc(meta)
    runner = _Runner(nc, NC)
    in_maps = _make_in_maps(inputs, meta)
    args = runner.prepare(in_maps)
    return {"meta": meta, "nc": nc, "runner": runner, "args": args}


def kernel(**inputs):
    st = build_state(**inputs)
    outs = st["runner"].run(st["args"])
    return st["runner"].result0(outs)


# revision 23
# speedup vs baseline: 506.7695x; 506.7695x over previous
"""GAT 3-layer multi-head GNN (residual+BN) on 8 TRN2 NeuronCores via Bass.

Sharding: nodes + incoming edges partitioned by dst across 8 cores; weights
replicated; per-layer node-feature tables in DRAM gathered per edge with
dma_gather (int16 idx, 4 src-chunks of 25088 rows); segment softmax and
aggregation via DVE iota-compare masks + TensorE segment matmuls into PSUM
(no scatter); AllGather of node features between layers; AllReduce pooling.

Edge order per core: (src_chunk, dst_window, dst). Per-(chunk,window) group
sizes are padded to the max across cores so one static program serves all 8
cores (SPMD). All index prep is host-side integer work; float math on device.
"""
import sys
import numpy as np

sys.path.insert(0, "/opt/trn_rl_repo")

N, E, G, H = 100000, 1600000, 256, 2
NC = 8
NPC = N // NC                 # 12500
NWIN = 98
NPCP = NWIN * 128             # 12544
NTAB = NC * NPCP              # 100352
NCHUNK = 4
CHROWS = NTAB // NCHUNK       # 25088
GCALL = 1024                  # idx per dma_gather call
STE = 6 * GCALL               # supertile edges (48 blocks)
EPS = 1e-5

LAYERS = [  # (F_in, C, table_cols)
    (5, 32, 128),
    (64, 64, 256),
    (128, 64, 256),
]

_STATE = {}


def _wrap16(idx2d):
    nc_, n = idx2d.shape
    w = idx2d.reshape(nc_, n // 16, 16).transpose(0, 2, 1)
    return np.ascontiguousarray(np.tile(w, (1, 8, 1)).astype(np.int16))


def _host_prep(edge_index, batch):
    src = np.concatenate([edge_index[0].astype(np.int64), np.arange(N, dtype=np.int64)])
    dst = np.concatenate([edge_index[1].astype(np.int64), np.arange(N, dtype=np.int64)])
    core = dst // NPC
    dst_local = dst - core * NPC
    src_perm = src + 44 * (src // NPC)

    per_core = []
    cnt = np.zeros((NC, NCHUNK, NWIN), np.int64)
    for k in range(NC):
        sel = core == k
        s, dl = src_perm[sel], dst_local[sel]
        ch = s // CHROWS
        w = dl // 128
        order = np.lexsort((dl, w, ch))
        s, dl, ch, w = s[order], dl[order], ch[order], w[order]
        per_core.append((s, dl, ch, w))
        np.add.at(cnt[k], (ch, w), 1)

    gsz = ((cnt.max(axis=0) + 127) // 128) * 128        # [NCHUNK, NWIN]
    chunk_tot = gsz.sum(axis=1)
    chunk_pad = ((chunk_tot + STE - 1) // STE) * STE
    tail = chunk_pad - chunk_tot
    epad = int(chunk_pad.sum())

    idx16 = np.zeros((NC, epad), np.int16)
    dstrel = np.full((NC, epad), -1.0, np.float32)
    for k in range(NC):
        s, dl, ch, w = per_core[k]
        pos = ptr = 0
        for c in range(NCHUNK):
            for wi in range(NWIN):
                g, n = int(gsz[c, wi]), int(cnt[k, c, wi])
                idx16[k, pos:pos + n] = (s[ptr:ptr + n] - c * CHROWS).astype(np.int16)
                dstrel[k, pos:pos + n] = (dl[ptr:ptr + n] - 128 * wi).astype(np.float32)
                ptr += n
                pos += g
            pos += int(tail[c])
        assert ptr == len(s) and pos == epad

    # group schedule: (window, n_blocks) in edge order; chunk tails are inert
    # groups on the last window (masks all-zero there).
    groups = []
    st_chunk = []                      # chunk id per supertile
    for c in range(NCHUNK):
        for wi in range(NWIN):
            if gsz[c, wi]:
                groups.append((wi, int(gsz[c, wi]) // 128))
        if tail[c]:
            groups.append((NWIN - 1, int(tail[c]) // 128))
        st_chunk += [c] * (int(chunk_pad[c]) // STE)

    # per-block: (window, is_first_of_group, is_last_of_group)
    blocksched = []
    for wi, nb in groups:
        for j in range(nb):
            blocksched.append((wi, j == 0, j == nb - 1))
    assert len(blocksched) == epad // 128

    batchv = np.full((NC, NPCP), -1.0, np.float32)
    b = np.asarray(batch, np.int64)
    for k in range(NC):
        batchv[k, :NPC] = b[k * NPC:(k + 1) * NPC].astype(np.float32)

    import ml_dtypes
    meta_drrep = np.repeat(dstrel.astype(ml_dtypes.bfloat16)[:, None, :], 128, axis=1)
    return {
        "drrep": meta_drrep,
        "epad": epad,
        "st_chunk": st_chunk,
        "blocksched": blocksched,
        "idx16w": _wrap16(idx16),
        "dstrel_col": np.ascontiguousarray(
            dstrel.reshape(NC, epad // 128, 128).transpose(0, 2, 1)),   # [NC,128,nblk]
        "batchv": np.ascontiguousarray(
            batchv.reshape(NC, NWIN, 128).transpose(0, 2, 1)),           # [NC,128,NWIN]
    }


def _build_nc(meta):
    from concourse import bass, bacc, mybir
    import concourse.tile as tile
    from concourse.masks import make_identity

    F32, BF16, I16 = mybir.dt.float32, mybir.dt.bfloat16, mybir.dt.int16
    AOP = mybir.AluOpType
    AF = mybir.ActivationFunctionType

    epad = meta["epad"]
    nblk_tot = epad // 128
    nst = epad // STE
    st_chunk = meta["st_chunk"]
    blocksched = meta["blocksched"]

    nc = bacc.Bacc("TRN2", target_bir_lowering=False, debug=False, num_devices=NC)

    def inp(name, shape, dt=F32):
        return nc.dram_tensor(name, shape, dt, kind="ExternalInput")

    xT = inp("xT", [5, NTAB])
    xTown = inp("xTown", [5, NPCP])
    idx16_d = inp("idx16", [128, epad // 16], I16)
    drcol_d = inp("drcol", [128, nblk_tot])
    drrep_d = inp("drrep", [128, epad], BF16)
    batchv_d = inp("batchv", [128, NWIN])
    iota_col_d = inp("iota_col", [128, 1])
    iota_mat_d = inp("iota_mat", [128, GCALL], BF16)

    ws = {}
    for l in (1, 2, 3):
        Fi, C, TC = LAYERS[l - 1]
        ws[f"w{l}"] = inp(f"w{l}", [Fi, 2 * C])
        ws[f"w{l}T"] = inp(f"w{l}T", [2 * C, Fi])
        ws[f"ast{l}"] = inp(f"ast{l}", [2 * C, 2])   # [:,0]=a_src heads stacked, [:,1]=a_dst
        for nm in ("b", "g", "be", "m", "v", "pb"):
            ws[f"{nm}{l}"] = inp(f"{nm}{l}", [128, 2 * C])
        ws[f"pw{l}"] = inp(f"pw{l}", [Fi, 2 * C])
    ws["fw"] = inp("fw", [128, 32])
    for nm, d in (("fb", 32), ("g4", 32), ("be4", 32), ("m4", 32), ("v4", 32),
                  ("l1b", 32), ("l2b", 10)):
        ws[nm] = inp(nm, [128, d])
    ws["l1w"] = inp("l1w", [32, 32])
    ws["l2w"] = inp("l2w", [32, 10])

    out_d = nc.dram_tensor("out", [256, 10], F32, kind="ExternalOutput")

    xTbf_d = nc.dram_tensor("xTbf", [5, NTAB], BF16)
    xTownbf_d = nc.dram_tensor("xTownbf", [5, NPCP], BF16)
    table_d = nc.dram_tensor("table", [NTAB, 256], BF16)
    shard_d = nc.dram_tensor("shard", [128, NPCP], BF16)
    gath_d = nc.dram_tensor("gath", [128 * NC, NPCP], BF16, addr_space="Shared")
    poolin_d = nc.dram_tensor("poolin", [128, 2, 132], F32)
    poolout_d = nc.dram_tensor("poolout", [128, 2, 132], F32, addr_space="Shared")
    proj_d = nc.dram_tensor("proj", [128, NWIN * 128], BF16)

    def bc(ap, shape):
        if ap.shape[0] == 1 and shape[0] != 1:
            ap = ap.partition_broadcast(shape[0]).squeeze()
        while ap.ndim < len(shape):
            ap = ap.unsqueeze(1)
        return ap.to_broadcast(shape) if list(ap.shape) != list(shape) else ap

    with tile.TileContext(nc) as tc:
        with tc.tile_pool(name="const", bufs=1) as cpool, \
             tc.tile_pool(name="accp", bufs=1) as apool, \
             tc.tile_pool(name="gat", bufs=2) as gpool, \
             tc.tile_pool(name="msk", bufs=2) as mpool, \
             tc.tile_pool(name="sml", bufs=3) as spool, \
             tc.tile_pool(name="wts", bufs=1) as wpool, \
             tc.tile_pool(name="tbl", bufs=2) as tpool, \
             tc.tile_pool(name="psA", bufs=3, space="PSUM") as psA, \
             tc.tile_pool(name="psB", bufs=3, space="PSUM") as psB, \
             tc.tile_pool(name="psC", bufs=2, space="PSUM") as psC:

            ident = cpool.tile([128, 128], F32, tag="ident")
            make_identity(nc, ident[:])
            identb = cpool.tile([128, 128], BF16, tag="identb")
            nc.vector.tensor_copy(out=identb[:], in_=ident[:])
            iota_col = cpool.tile([128, 1], F32, tag="iotac")
            nc.sync.dma_start(out=iota_col[:], in_=iota_col_d.ap())
            iota_mat = cpool.tile([128, GCALL], BF16, tag="iotam")
            nc.sync.dma_start(out=iota_mat[:], in_=iota_mat_d.ap())

            drcol = cpool.tile([128, nblk_tot], F32, tag="drcol")
            nc.sync.dma_start(out=drcol[:], in_=drcol_d.ap())
            batchv = cpool.tile([128, NWIN], F32, tag="batchv")
            nc.sync.dma_start(out=batchv[:], in_=batchv_d.ap())

            acc = apool.tile([128, NWIN, 132], F32, tag="acc")
            xsh = apool.tile([128, NWIN, 132], BF16, tag="xsh")
            c_sb = apool.tile([128, NWIN, 2], BF16, tag="c_sb")

            # f32->bf16 casts of x inputs (SWDGE cast-DMA)
            nc.gpsimd.dma_start(out=xTbf_d.ap(), in_=xT.ap())
            nc.gpsimd.dma_start(out=xTownbf_d.ap(), in_=xTown.ap())

            for l in (1, 2, 3):
                Fi, C, TC = LAYERS[l - 1]
                Fo = 2 * C

                # ---- weights: W_aug bf16 [Fi, Fo+4], proj bf16, bn vectors ----
                wfull = wpool.tile([128, 260], F32, tag="waug")
                nc.sync.dma_start(out=wfull[:Fi, 0:Fo], in_=ws[f"w{l}"].ap())
                wT = wpool.tile([128, 128], F32, tag="wT")
                nc.sync.dma_start(out=wT[:Fo, 0:Fi], in_=ws[f"w{l}T"].ap())
                aT = wpool.tile([128, 2], F32, tag="aT")
                nc.sync.dma_start(out=aT[:Fo, 0:2], in_=ws[f"ast{l}"].ap())
                ps_v = psC.tile([128, 4], F32, tag="ps_small")
                for j in range(2):          # 0: a_src, 1: a_dst
                    for h in range(H):
                        nc.tensor.matmul(
                            out=ps_v[:Fi, 2 * j + h:2 * j + h + 1],
                            lhsT=wT[h * C:(h + 1) * C, 0:Fi],
                            rhs=aT[h * C:(h + 1) * C, j:j + 1],
                            start=True, stop=True)
                nc.vector.tensor_copy(out=wfull[:Fi, Fo:Fo + 4], in_=ps_v[:Fi, :])
                wbf = wpool.tile([128, 260], BF16, tag="wbf")
                nc.vector.tensor_copy(out=wbf[:Fi, 0:Fo + 4], in_=wfull[:Fi, 0:Fo + 4])
                pwbf = wpool.tile([128, 256], BF16, tag="pwbf")
                pwf = wpool.tile([128, 256], F32, tag="pwf")
                nc.sync.dma_start(out=pwf[:Fi, 0:Fo], in_=ws[f"pw{l}"].ap())
                nc.vector.tensor_copy(out=pwbf[:Fi, 0:Fo], in_=pwf[:Fi, 0:Fo])

                # bn affine: s = g / sqrt(v+eps); b2 = be - s*m  (+ gat bias b)
                bnrow = wpool.tile([128, 256], F32, tag="bnrow")     # [s | b2]
                vrow = spool.tile([128, 128], F32, tag="vrow")
                nc.sync.dma_start(out=vrow[:, 0:Fo], in_=ws[f"v{l}"].ap())
                nc.vector.tensor_scalar(vrow[:, 0:Fo], vrow[:, 0:Fo], EPS, None, AOP.add)
                nc.scalar.activation(out=vrow[:, 0:Fo], in_=vrow[:, 0:Fo], func=AF.Sqrt)
                nc.vector.reciprocal(out=bnrow[:, 0:Fo], in_=vrow[:, 0:Fo])
                grow = spool.tile([128, 128], F32, tag="grow")
                nc.sync.dma_start(out=grow[:, 0:Fo], in_=ws[f"g{l}"].ap())
                nc.vector.tensor_tensor(out=bnrow[:, 0:Fo], in0=bnrow[:, 0:Fo],
                                        in1=grow[:, 0:Fo], op=AOP.mult)
                mrow = spool.tile([128, 128], F32, tag="mrow")
                nc.sync.dma_start(out=mrow[:, 0:Fo], in_=ws[f"m{l}"].ap())
                # m' = m - b  (gat bias folds into bn input: bn(x+b) = s*x + (be - s*(m-b)))
                brow = spool.tile([128, 128], F32, tag="brow")
                nc.sync.dma_start(out=brow[:, 0:Fo], in_=ws[f"b{l}"].ap())
                nc.vector.tensor_tensor(out=mrow[:, 0:Fo], in0=mrow[:, 0:Fo],
                                        in1=brow[:, 0:Fo], op=AOP.subtract)
                nc.vector.tensor_tensor(out=mrow[:, 0:Fo], in0=mrow[:, 0:Fo],
                                        in1=bnrow[:, 0:Fo], op=AOP.mult)
                berow = spool.tile([128, 128], F32, tag="berow")
                nc.sync.dma_start(out=berow[:, 0:Fo], in_=ws[f"be{l}"].ap())
                nc.vector.tensor_tensor(out=bnrow[:, 128:128 + Fo], in0=berow[:, 0:Fo],
                                        in1=mrow[:, 0:Fo], op=AOP.subtract)
                pbrow = wpool.tile([128, 256], F32, tag="pbrow")
                nc.sync.dma_start(out=pbrow[:, 0:Fo], in_=ws[f"pb{l}"].ap())

                # ---- node table build ----
                for cs in range(NC):
                    for tb in range(7):       # 7 * 14 = 98 windows
                        xsl = tpool.tile([128, 14 * 128], BF16, tag="xsl")
                        c0 = tb * 14 * 128
                        if l == 1:
                            nc.sync.dma_start(
                                out=xsl[:5, :],
                                in_=xTbf_d.ap()[:, cs * NPCP + c0: cs * NPCP + c0 + 14 * 128])
                        else:
                            nc.sync.dma_start(
                                out=xsl[:Fi, :],
                                in_=gath_d.ap()[cs * 128:cs * 128 + Fi,
                                                c0:c0 + 14 * 128])
                        for nb in range(14):
                            ps_h = psA.tile([128, 260], F32, tag="ps_h")
                            nc.tensor.matmul(
                                out=ps_h[:, 0:Fo + 4],
                                lhsT=xsl[:Fi, nb * 128:(nb + 1) * 128],
                                rhs=wbf[:Fi, 0:Fo + 4], start=True, stop=True)
                            hrow = tpool.tile([128, 256], BF16, tag="hrow")
                            nc.scalar.copy(out=hrow[:, 0:Fo + 2], in_=ps_h[:, 0:Fo + 2])
                            r0 = cs * NPCP + c0 + nb * 128
                            nc.sync.dma_start(
                                out=table_d.ap()[r0:r0 + 128, 0:Fo + 2],
                                in_=hrow[:, 0:Fo + 2])

                # ---- own shard: e_dst cols + residual projection ----
                for tb in range(7):
                    xsl = tpool.tile([128, 14 * 128], BF16, tag="xslo")
                    c0 = tb * 14 * 128
                    src_ap = (xTownbf_d if l == 1 else shard_d).ap()
                    nc.sync.dma_start(out=xsl[:Fi, :], in_=src_ap[0:Fi, c0:c0 + 14 * 128])
                    for nb in range(14):
                        nwb = tb * 14 + nb
                        ps_h = psA.tile([128, 260], F32, tag="ps_h")
                        nc.tensor.matmul(out=ps_h[:, 0:Fo + 4],
                                         lhsT=xsl[:Fi, nb * 128:(nb + 1) * 128],
                                         rhs=wbf[:Fi, 0:Fo + 4], start=True, stop=True)
                        nc.vector.tensor_copy(out=c_sb[:, nwb, :], in_=ps_h[:, Fo + 2:Fo + 4])
                        ps_p = psA.tile([128, 260], F32, tag="ps_h", name="ps_p")
                        nc.tensor.matmul(out=ps_p[:, 0:Fo],
                                         lhsT=xsl[:Fi, nb * 128:(nb + 1) * 128],
                                         rhs=pwbf[:Fi, 0:Fo], start=True, stop=True)
                        prt = tpool.tile([128, 128], BF16, tag="prt")
                        nc.scalar.copy(out=prt[:, 0:Fo], in_=ps_p[:, 0:Fo])
                        nc.sync.dma_start(out=proj_d.ap()[:, nwb * 128:nwb * 128 + Fo],
                                          in_=prt[:, 0:Fo])

                nc.vector.memset(acc[:, :, 0:132], 0.0)

                # ---- edge pipeline ----
                blk = 0
                ps_seg = None
                for st in range(nst):
                    ch = st_chunk[st]
                    hg = gpool.tile([128, 48, TC], BF16, tag="hg")
                    idxt = spool.tile([128, STE // 16], I16, tag="idxt")
                    nc.sync.dma_start(
                        out=idxt[:],
                        in_=idx16_d.ap()[:, st * (STE // 16):(st + 1) * (STE // 16)])
                    ps_c = psC.tile([128, 48, 2], F32, tag="ps_small")
                    for t in range(6):
                        eb = st * STE + t * GCALL
                        nc.gpsimd.dma_gather(
                            out_ap=hg[:, 8 * t:8 * t + 8, :],
                            in_ap=table_d.ap()[ch * CHROWS:(ch + 1) * CHROWS, 0:TC],
                            idxs_ap=idxt[:, t * 64:(t + 1) * 64],
                            num_idxs=GCALL, num_idxs_reg=GCALL, elem_size=TC, elem_step=256)
                        drrep = mpool.tile([128, GCALL], BF16, tag="drrep")
                        nc.sync.dma_start(out=drrep[:], in_=drrep_d.ap()[:, eb:eb + GCALL])
                        m_dT = mpool.tile([128, 8, 128], BF16, tag="mdT")
                        nc.vector.tensor_scalar(
                            m_dT[:].rearrange("p a b -> p (a b)"),
                            drrep[:], iota_col[:], None, AOP.is_equal)
                        for jj in range(8):
                            j = 8 * t + jj
                            nc.tensor.matmul(out=ps_c[:, j, :], lhsT=m_dT[:, jj, :],
                                             rhs=c_sb[:, blocksched[blk + j][0], :],
                                             start=True, stop=True)
                    alf = spool.tile([128, 48, 2], F32, tag="alf")
                    nc.vector.tensor_copy(out=alf[:], in_=hg[:, :, Fo:Fo + 2])
                    nc.vector.tensor_tensor(out=alf[:], in0=alf[:], in1=ps_c[:], op=AOP.add)
                    nc.scalar.activation(out=alf[:], in_=alf[:], func=AF.Lrelu, alpha=0.2)
                    nc.scalar.activation(out=alf[:], in_=alf[:], func=AF.Exp)
                    nc.vector.tensor_copy(out=hg[:, :, Fo:Fo + 2], in_=alf[:])
                    for h in range(H):
                        nc.vector.tensor_tensor(
                            out=hg[:, :, h * C:(h + 1) * C],
                            in0=hg[:, :, h * C:(h + 1) * C],
                            in1=hg[:, :, Fo + h:Fo + h + 1].to_broadcast([128, 48, C]),
                            op=AOP.mult)
                    for t in range(6):
                        m_ed = mpool.tile([128, 8, 128], BF16, tag="med")
                        nc.vector.tensor_tensor(
                            out=m_ed[:],
                            in0=iota_mat[:].rearrange("p (a b) -> p a b", b=128),
                            in1=drcol[:, st * 48 + 8 * t:st * 48 + 8 * t + 8]
                                .unsqueeze(-1).to_broadcast([128, 8, 128]),
                            op=AOP.is_equal)
                        for jj in range(8):
                            j = 8 * t + jj
                            wi, first, last = blocksched[blk + j]
                            if first:
                                ps_seg = psB.tile([128, 132], F32, tag="ps_seg")
                            nc.tensor.matmul(out=ps_seg[:, 0:Fo + 2], lhsT=m_ed[:, jj, :],
                                             rhs=hg[:, j, 0:Fo + 2], start=first, stop=last)
                            if last:
                                nc.vector.tensor_tensor(
                                    out=acc[:, wi, 0:Fo + 2], in0=acc[:, wi, 0:Fo + 2],
                                    in1=ps_seg[:, 0:Fo + 2], op=AOP.add)
                    blk += 48

                # ---- finalize: normalize, bn, relu, +proj ----
                nc.vector.tensor_scalar(acc[:, :, Fo:Fo + 2], acc[:, :, Fo:Fo + 2],
                                        1e-16, None, AOP.add)
                rc = spool.tile([128, NWIN, 2], F32, tag="rc")
                nc.vector.reciprocal(out=rc[:], in_=acc[:, :, Fo:Fo + 2])
                for h in range(H):
                    nc.vector.tensor_tensor(
                        out=acc[:, :, h * C:(h + 1) * C],
                        in0=acc[:, :, h * C:(h + 1) * C],
                        in1=rc[:, :, h:h + 1].to_broadcast([128, NWIN, C]), op=AOP.mult)
                nc.vector.tensor_tensor(
                    out=acc[:, :, 0:Fo], in0=acc[:, :, 0:Fo],
                    in1=bnrow[:, 0:Fo].unsqueeze(1).to_broadcast([128, NWIN, Fo]), op=AOP.mult)
                nc.vector.tensor_tensor(
                    out=acc[:, :, 0:Fo], in0=acc[:, :, 0:Fo],
                    in1=bnrow[:, 128:128 + Fo].unsqueeze(1).to_broadcast([128, NWIN, Fo]), op=AOP.add)
                nc.scalar.activation(out=xsh[:, :, 0:Fo], in_=acc[:, :, 0:Fo], func=AF.Relu)
                for hf in range(2):
                    nw2 = NWIN // 2
                    projt = gpool.tile([128, nw2, 128], BF16, tag="hg", name=f"projt{l}{hf}")
                    nc.sync.dma_start(
                        out=projt[:].rearrange("p a b -> p (a b)"),
                        in_=proj_d.ap()[:, hf * nw2 * 128:(hf + 1) * nw2 * 128])
                    nc.vector.tensor_tensor(
                        out=xsh[:, hf * nw2:(hf + 1) * nw2, 0:Fo],
                        in0=xsh[:, hf * nw2:(hf + 1) * nw2, 0:Fo],
                        in1=projt[:, :, 0:Fo], op=AOP.add)
                nc.vector.tensor_tensor(
                    out=xsh[:, :, 0:Fo], in0=xsh[:, :, 0:Fo],
                    in1=pbrow[:, 0:Fo].unsqueeze(1).to_broadcast([128, NWIN, Fo]), op=AOP.add)

                if l < 3:
                    # transpose shard to feat-major, ship, all-gather
                    for nb in range(NWIN):
                        ps_t = psA.tile([128, 260], BF16, tag="ps_h", name="ps_tb")
                        nc.tensor.matmul(out=ps_t[:, 0:128], lhsT=xsh[:, nb, 0:128],
                                         rhs=identb[:], is_transpose=True, start=True, stop=True)
                        tt = tpool.tile([128, 128], BF16, tag="tt")
                        nc.scalar.copy(out=tt[:Fo, :], in_=ps_t[0:Fo, 0:128])
                        nc.sync.dma_start(out=shard_d.ap()[0:Fo, nb * 128:(nb + 1) * 128],
                                          in_=tt[:Fo, :])
                    nc.gpsimd.collective_compute(
                        "AllGather", AOP.bypass, replica_groups=[list(range(NC))],
                        ins=[shard_d.ap()], outs=[gath_d.ap()])

            # ---------- pooling + MLP head ----------
            Fo = 256 if False else 128
            nc.vector.memset(xsh[:, :, 128:129], 1.0)
            bm1 = spool.tile([128, NWIN], F32, tag="bm1")
            nc.vector.tensor_scalar(bm1[:], batchv[:], -128.0, None, AOP.add)
            ps_pool = [psB.tile([128, 132], F32, name=f"pspool{_g}", tag="ps_seg") for _g in range(2)]
            for nb in range(NWIN):
                for gh in range(2):
                    gm = spool.tile([128, 128], BF16, tag="gm")
                    nc.vector.tensor_scalar(
                        gm[:], iota_mat[:, 0:128],
                        (batchv if gh == 0 else bm1)[:, nb:nb + 1], None, AOP.is_equal)
                    nc.tensor.matmul(out=ps_pool[gh][:, 0:129], lhsT=gm[:],
                                     rhs=xsh[:, nb, 0:129],
                                     start=(nb == 0), stop=(nb == NWIN - 1))
            plin = spool.tile([128, 2, 132], F32, tag="plin")
            for gh in range(2):
                nc.vector.tensor_copy(out=plin[:, gh, 0:129], in_=ps_pool[gh][:, 0:129])
            nc.sync.dma_start(out=poolin_d.ap().rearrange("p a f -> p (a f)"),
                              in_=plin[:].rearrange("p a f -> p (a f)"))
            nc.gpsimd.collective_compute(
                "AllReduce", AOP.add, replica_groups=[list(range(NC))],
                ins=[poolin_d.ap()], outs=[poolout_d.ap()])
            pl = spool.tile([128, 2, 132], F32, tag="pl")
            nc.sync.dma_start(out=pl[:].rearrange("p a f -> p (a f)"),
                              in_=poolout_d.ap().rearrange("p a f -> p (a f)"))
            # mean
            cnt_t = spool.tile([128, 2, 1], F32, tag="cnt")
            nc.vector.tensor_scalar(cnt_t[:].rearrange("p a b -> p (a b)"),
                                    pl[:, :, 128:129].rearrange("p a b -> p (a b)"),
                                    1.0, None, AOP.max)
            rcc = spool.tile([128, 2, 1], F32, tag="rcc")
            nc.vector.reciprocal(out=rcc[:].rearrange("p a b -> p (a b)"),
                                 in_=cnt_t[:].rearrange("p a b -> p (a b)"))
            nc.vector.tensor_tensor(out=pl[:, :, 0:128], in0=pl[:, :, 0:128],
                                    in1=rcc[:].to_broadcast([128, 2, 128]), op=AOP.mult)

            # z1 = relu(bn4(pooled @ fw + fb))
            fw_t = spool.tile([128, 32], F32, tag="fw")
            nc.sync.dma_start(out=fw_t[:], in_=ws["fw"].ap())
            z1 = spool.tile([128, 2, 32], F32, tag="z1")
            for gh in range(2):
                ps_t = psA.tile([128, 260], F32, tag="ps_h", name="ps_plT")
                nc.tensor.matmul(out=ps_t[:, 0:128], lhsT=pl[:, gh, 0:128], rhs=ident[:],
                                 is_transpose=True, start=True, stop=True)
                plT = spool.tile([128, 128], F32, tag="plT")
                nc.vector.tensor_copy(out=plT[:], in_=ps_t[:, 0:128])
                ps_z = psC.tile([128, 48, 2], F32, tag="ps_small", name="ps_z1")
                nc.tensor.matmul(out=ps_z[:].rearrange("p a b -> p (a b)")[0:128, 0:32],
                                 lhsT=plT[:], rhs=fw_t[:], start=True, stop=True)
                nc.vector.tensor_copy(out=z1[:, gh, :],
                                      in_=ps_z[:].rearrange("p a b -> p (a b)")[:, 0:32])
            # bn4 affine on device
            s4 = spool.tile([128, 32], F32, tag="s4")
            b4 = spool.tile([128, 32], F32, tag="b4")
            v4r = spool.tile([128, 32], F32, tag="v4r")
            nc.sync.dma_start(out=v4r[:], in_=ws["v4"].ap())
            nc.vector.tensor_scalar(v4r[:], v4r[:], EPS, None, AOP.add)
            nc.scalar.activation(out=v4r[:], in_=v4r[:], func=AF.Sqrt)
            nc.vector.reciprocal(out=s4[:], in_=v4r[:])
            g4r = spool.tile([128, 32], F32, tag="g4r")
            nc.sync.dma_start(out=g4r[:], in_=ws["g4"].ap())
            nc.vector.tensor_tensor(out=s4[:], in0=s4[:], in1=g4r[:], op=AOP.mult)
            m4r = spool.tile([128, 32], F32, tag="m4r")
            nc.sync.dma_start(out=m4r[:], in_=ws["m4"].ap())
            fbr = spool.tile([128, 32], F32, tag="fbr")
            nc.sync.dma_start(out=fbr[:], in_=ws["fb"].ap())
            nc.vector.tensor_tensor(out=m4r[:], in0=m4r[:], in1=fbr[:], op=AOP.subtract)
            nc.vector.tensor_tensor(out=m4r[:], in0=m4r[:], in1=s4[:], op=AOP.mult)
            be4r = spool.tile([128, 32], F32, tag="be4r")
            nc.sync.dma_start(out=be4r[:], in_=ws["be4"].ap())
            nc.vector.tensor_tensor(out=b4[:], in0=be4r[:], in1=m4r[:], op=AOP.subtract)
            nc.vector.tensor_tensor(out=z1[:], in0=z1[:],
                                    in1=s4[:].unsqueeze(1).to_broadcast([128, 2, 32]), op=AOP.mult)
            nc.vector.tensor_tensor(out=z1[:], in0=z1[:],
                                    in1=b4[:].unsqueeze(1).to_broadcast([128, 2, 32]), op=AOP.add)
            nc.scalar.activation(out=z1[:], in_=z1[:], func=AF.Relu)

            # z2 = relu(z1 @ l1w + l1b); out = z2 @ l2w + l2b
            def mlp_step(zin, wname, bname, dout, relu):
                wt = spool.tile([32, 32], F32, tag="wmlp")
                nc.sync.dma_start(out=wt[:32, 0:dout], in_=ws[wname].ap())
                br = spool.tile([128, 32], F32, tag="bmlp")
                nc.sync.dma_start(out=br[:, 0:dout], in_=ws[bname].ap())
                zo = spool.tile([128, 2, 32], F32, tag="zo")
                for gh in range(2):
                    ps_t = psA.tile([128, 260], F32, tag="ps_h", name="ps_mlpT")
                    nc.tensor.matmul(out=ps_t[0:32, 0:128], lhsT=zin[:, gh, 0:32],
                                     rhs=ident[:], is_transpose=True, start=True, stop=True)
                    zT = spool.tile([32, 128], F32, tag="zT")
                    nc.vector.tensor_copy(out=zT[:], in_=ps_t[0:32, 0:128])
                    ps_z = psC.tile([128, 48, 2], F32, tag="ps_small", name="ps_z2")
                    nc.tensor.matmul(out=ps_z[:].rearrange("p a b -> p (a b)")[0:128, 0:dout],
                                     lhsT=zT[:], rhs=wt[:32, 0:dout], start=True, stop=True)
                    nc.vector.tensor_copy(out=zo[:, gh, 0:dout],
                                          in_=ps_z[:].rearrange("p a b -> p (a b)")[:, 0:dout])
                nc.vector.tensor_tensor(out=zo[:, :, 0:dout], in0=zo[:, :, 0:dout],
                                        in1=br[:, 0:dout].unsqueeze(1).to_broadcast([128, 2, dout]),
                                        op=AOP.add)
                if relu:
                    nc.scalar.activation(out=zo[:, :, 0:dout], in_=zo[:, :, 0:dout], func=AF.Relu)
                return zo

            z2 = mlp_step(z1, "l1w", "l1b", 32, True)
            z3 = mlp_step(z2, "l2w", "l2b", 10, False)
            for gh in range(2):
                nc.sync.dma_start(out=out_d.ap()[gh * 128:(gh + 1) * 128, :],
                                  in_=z3[:, gh, 0:10])

    return nc


def _make_in_maps(inputs, meta):
    x = np.asarray(inputs["x"], np.float32)
    xTp = np.zeros((5, NTAB), np.float32)
    xp = x.reshape(NC, NPC, 5)
    for k in range(NC):
        xTp[:, k * NPCP:k * NPCP + NPC] = xp[k].T
    iota_col = np.arange(128, dtype=np.float32).reshape(128, 1)
    import ml_dtypes
    iota_mat = np.tile(np.arange(128, dtype=np.float32), GCALL // 128).reshape(1, GCALL)
    iota_mat = np.repeat(iota_mat, 128, axis=0).astype(ml_dtypes.bfloat16)

    common = {"xT": xTp, "iota_col": iota_col, "iota_mat": iota_mat}
    for l in (1, 2, 3):
        w = np.asarray(inputs[f"w{l}"], np.float32)
        common[f"w{l}"] = w
        common[f"w{l}T"] = np.ascontiguousarray(w.T)
        a_s = np.asarray(inputs[f"as{l}"], np.float32).reshape(-1, 1)
        a_d = np.asarray(inputs[f"ad{l}"], np.float32).reshape(-1, 1)
        common[f"ast{l}"] = np.ascontiguousarray(np.concatenate([a_s, a_d], axis=1))
        for nm in ("b", "g", "be", "m", "v"):
            common[f"{nm}{l}"] = np.repeat(
                np.asarray(inputs[f"{nm}{l}"], np.float32).reshape(1, -1), 128, axis=0)
        common[f"pw{l}"] = np.asarray(inputs[f"p{l}w"], np.float32)
        common[f"pb{l}"] = np.repeat(
            np.asarray(inputs[f"p{l}b"], np.float32).reshape(1, -1), 128, axis=0)
    common["fw"] = np.asarray(inputs["fw"], np.float32)
    for nm, src in (("fb", "fb"), ("g4", "g4"), ("be4", "be4"), ("m4", "m4"),
                    ("v4", "v4"), ("l1b", "l1b"), ("l2b", "l2b")):
        common[nm] = np.repeat(np.asarray(inputs[src], np.float32).reshape(1, -1), 128, axis=0)
    common["l1w"] = np.asarray(inputs["l1w"], np.float32)
    common["l2w"] = np.asarray(inputs["l2w"], np.float32)

    in_maps = []
    for k in range(NC):
        m = dict(common)
        m["xTown"] = np.ascontiguousarray(xTp[:, k * NPCP:(k + 1) * NPCP])
        m["idx16"] = meta["idx16w"][k]
        m["drcol"] = meta["dstrel_col"][k]
        m["drrep"] = meta["drrep"][k]
        m["batchv"] = meta["batchv"][k]
        in_maps.append(m)
    return in_maps


class _Runner:
    def __init__(self, nc, n_cores):
        import jax
        from jax.sharding import Mesh, PartitionSpec
        from jax.experimental.shard_map import shard_map
        from concourse import bass2jax, mybir

        bass2jax.install_neuronx_cc_hook()
        nc.finalize()
        self.n_cores = n_cores
        partition_name = nc.partition_id_tensor.name if nc.partition_id_tensor else None
        in_names, out_names, out_avals, zero_outs = [], [], [], []
        for alloc in nc.m.functions[0].allocations:
            if not isinstance(alloc, mybir.MemoryLocationSet):
                continue
            name = alloc.memorylocations[0].name
            if alloc.kind == "ExternalInput":
                if name != partition_name:
                    in_names.append(name)
            elif alloc.kind == "ExternalOutput":
                shape = tuple(alloc.tensor_shape)
                dtype = mybir.dt.np(alloc.dtype)
                out_names.append(name)
                out_avals.append(jax.core.ShapedArray(shape, dtype))
                zero_outs.append(np.zeros(shape, dtype))
        self.in_names, self.out_names = in_names, out_names
        self.out_avals = out_avals
        n_params, n_outs = len(in_names), len(out_avals)
        self.n_params = n_params
        all_in = list(in_names) + list(out_names)
        if partition_name is not None:
            all_in.append(partition_name)

        def _body(*args):
            operands = list(args)
            if partition_name is not None:
                operands.append(bass2jax.partition_id_tensor())
            return tuple(bass2jax._bass_exec_p.bind(
                *operands, out_avals=tuple(out_avals), in_names=tuple(all_in),
                out_names=tuple(out_names), lowering_input_output_aliases=(),
                sim_require_finite=False, sim_require_nnan=False, nc=nc))

        devices = jax.devices()[:n_cores]
        mesh = Mesh(np.asarray(devices), ("core",))
        self._mesh = mesh
        self.fn = jax.jit(
            shard_map(_body, mesh=mesh,
                      in_specs=(PartitionSpec("core"),) * (n_params + n_outs),
                      out_specs=(PartitionSpec("core"),) * n_outs, check_rep=False),
            keep_unused=True)
        self.zero_outs = zero_outs

    def prepare(self, in_maps):
        import jax
        from jax.sharding import NamedSharding, PartitionSpec
        sh = NamedSharding(self.fn.__wrapped__ if False else self._mesh, PartitionSpec("core"))
        per_core = [[np.asarray(m[nm]) for nm in self.in_names] for m in in_maps]
        args = [np.concatenate([per_core[c][i] for c in range(self.n_cores)], axis=0)
                for i in range(self.n_params)]
        args += [np.zeros((self.n_cores * z.shape[0], *z.shape[1:]), z.dtype)
                 for z in self.zero_outs]
        args = [jax.device_put(a, sh) for a in args]
        jax.block_until_ready(args)
        return args

    def run(self, args):
        import jax
        outs = self.fn(*args)
        jax.block_until_ready(outs)
        return outs

    def result0(self, outs, name="out"):
        i = self.out_names.index(name)
        a = np.asarray(outs[i])
        per = self.out_avals[i].shape[0]
        return a[:per]


def build_state(**inputs):
    meta = _host_prep(np.asarray(inputs["edge_index"]), np.asarray(inputs["batch"]))
    nc = _build_nc(meta)
    runner = _Runner(nc, NC)
    in_maps = _make_in_maps(inputs, meta)
    args = runner.prepare(in_maps)
    return {"meta": meta, "nc": nc, "runner": runner, "args": args}


def kernel(**inputs):
    st = build_state(**inputs)
    outs = st["runner"].run(st["args"])
    return st["runner"].result0(outs)
